# revision 14
# baseline (speedup 1.0000x reference)
"""Trainium2 Bass kernel for nn_CP_Based (CP-decomposition feature-product layer).

Math: out[b,u] = sum_r prod_f ( x0[b,f]*K[0,r,f,u] + x1[b,f]*K[1,r,f,u] )
  with x0 = 1/sqrt(1+X^2), x1 = X/sqrt(1+X^2).
Factor the normalization out of the f-product:
  out[b,u] = S[b] * sum_r prod_f ( K0[f,ru] + X[b,f]*K1[f,ru] ),
  S[b] = 1/sqrt(prod_f (1+X[b,f]^2)).
The 32-feature product is decomposed into 8 groups of 4 features. Each group's
product is a linear map from the 16 multilinear monomials of its 4 features:
  G[b, g, ru] = sum_m Q[b, g, m] * C[g, m, ru]
Layout: batch rows sit on the PARTITION axis of the matmul OUTPUT, so each
matmul is (stationary QT[m, b-chunk]) x (moving C-block[m, (g,ru)]):
  - Q [128b, (c,g,i,j)] built directly (no staging buffer) on DVE/GPSIMD,
    stored bf16; the ones-cells are memset once per rotating buffer
  - QT via one DMA-transpose instruction (no PE transpose, no PSUM evacuation)
  - 2 matmuls per 128-row chunk: groups 0-3 (K=64) into psum tile alpha bank c
    and groups 4-7 (K=64) into psum tile beta bank c; bf16 moving = 1 cyc/row
  - Act engine evacuates beta with ONE wide copy -> bcp (SBUF bf16); the DVE
    has a single PSUM read port, so level-1 chain mults are alpha(PSUM) x
    bcp(SBUF), split DVE/GPSIMD by chunk
  - remaining product levels in bf16 (DVE 2x mode), rank-sum via strided
    tensor_reduce (ru packed u-major: ru = u*10+r), then the fp32 S scale
  - S via Act Square + DVE (+1, prod-reduce, reciprocal) + Act Sqrt (Square
    and Sqrt share one act table set -> no per-macro table reloads)
The emission is software-pipelined: fronts run two macros ahead of backs so
no engine queue head-of-line blocks on the PSUM->chain dependency. X loads
and output stores are batched 4 macros per DMA to keep HWDGE occupancy low.

Sharding: pure data-parallel over batch: 131072 rows -> 8 cores x 16384.
"""

import sys

import numpy as np

sys.path.insert(0, "/opt/trn_rl_repo")

import concourse.bacc as bacc  # noqa: E402
import concourse.mybir as mybir  # noqa: E402
from concourse.bass_utils import run_bass_kernel_spmd  # noqa: E402
from concourse.tile import TileContext  # noqa: E402

F32 = mybir.dt.float32
BF16 = mybir.dt.bfloat16
FP16 = mybir.dt.float16
AF = mybir.ActivationFunctionType
OP = mybir.AluOpType
AX = mybir.AxisListType

B_FULL = 131072
N_CORES = 8
B_CORE = B_FULL // N_CORES  # 16384
F = 32
R, U = 10, 8
RU = R * U  # 80
NG = 8  # feature groups of 4
TILE_B = 128
CHUNK = 4  # 128-row chunks per macro tile
MACRO_B = TILE_B * CHUNK  # 512
N_MACRO = B_CORE // MACRO_B  # 32
CG = CHUNK * NG  # 32 (chunk, group) pairs
GRP = 4  # macros per X-load / out-store DMA
N_GRP = N_MACRO // GRP  # 8
NBUF = 5  # rotation depth of the q pool (ones-cells memset once per buffer)


def build_nc():
    nc = bacc.Bacc()
    X = nc.dram_tensor(
        "X", [N_GRP, TILE_B, GRP, CHUNK, F], F32, kind="ExternalInput"
    )
    # C rows: m = g*16 + i*4 + j; cols: g*80 + u*10 + r (within-half blocks)
    C = nc.dram_tensor("C", [128, 2 * 4 * RU], FP16, kind="ExternalInput")
    out = nc.dram_tensor(
        "out", [N_GRP, TILE_B, GRP, CHUNK, U], F32, kind="ExternalOutput"
    )

    with TileContext(nc) as tc:
        with (
            tc.tile_pool(name="const", bufs=1) as cpool,
            tc.tile_pool(name="xin", bufs=3) as xpool,
            tc.tile_pool(name="sno", bufs=5) as spool,
            tc.tile_pool(name="qq", bufs=NBUF) as qpool,
            tc.tile_pool(name="qt", bufs=5) as tpool,
            tc.tile_pool(name="bcp", bufs=2) as bpool,
            tc.tile_pool(name="chain", bufs=3) as lpool,
            tc.tile_pool(name="outp", bufs=3) as opool,
            tc.tile_pool(name="psum", bufs=1, space="PSUM") as pspool,
        ):
            c_sb = cpool.tile([128, 2 * 4 * RU], FP16, tag="c_sb")
            nc.sync.dma_start(out=c_sb[:], in_=C[:, :])

            state = {}  # macro index -> tiles needed by the back-end
            xg_tiles = {}

            def load_x(gi):
                xg_t = xpool.tile(
                    [TILE_B, GRP, CHUNK, F], F32, tag="x", name="xt"
                )
                nc.sync.dma_start(out=xg_t[:], in_=X[gi])
                xg_tiles[gi] = xg_t

            load_x(0)

            def front(mi):
                gi, k = divmod(mi, GRP)
                if k == 0 and gi + 1 < N_GRP:
                    load_x(gi + 1)  # prefetch next group
                xm = xg_tiles[gi][:, k]  # [128, CHUNK, F]

                # ---- S = 1/sqrt(prod_f (1 + x^2)); heavy ops on GPSIMD
                sq = spool.tile([TILE_B, CHUNK, F], F32, tag="sq")
                nc.gpsimd.tensor_tensor(sq[:], xm, xm, OP.mult)
                sp1 = spool.tile([TILE_B, CHUNK, F], F32, tag="sp1")
                nc.gpsimd.tensor_scalar_add(sp1[:], sq[:], 1.0)
                pr = spool.tile([TILE_B, CHUNK], F32, tag="pr")
                nc.vector.tensor_reduce(pr[:], sp1[:], AX.X, OP.mult)
                rp = spool.tile([TILE_B, CHUNK], F32, tag="rp")
                nc.vector.reciprocal(rp[:], pr[:])
                s_t = spool.tile([TILE_B, CHUNK], F32, tag="s_t")
                nc.scalar.activation(s_t[:], rp[:], AF.Sqrt, scale=2.0**-48)

                # ---- build Q[b, cg, i, j] (bf16) directly ----
                # cells: [0,0]=1 (memset, amortized), [0,1:4]=(Xc,Xd,XcXd),
                # [1:4,0]=(Xa,Xb,XaXb), [1:4,1:4]=outer of the two triples
                q = qpool.tile([TILE_B, CG, 4, 4], FP16, tag="q")
                if mi < NBUF:
                    nc.vector.memset(q[:, :, 0, 0], 1.0)
                xg = xm.rearrange("p c (g j) -> p (c g) j", j=4)
                # (Xc, Xd) and (Xa, Xb)
                nc.gpsimd.tensor_copy(q[:, :, 0, 1:3], xg[:, :, 2:4])
                nc.gpsimd.tensor_copy(q[:, :, 1:3, 0], xg[:, :, 0:2])
                # XcXd and XaXb (read fp32 x directly)
                nc.gpsimd.tensor_tensor(
                    q[:, :, 0, 3], xg[:, :, 2], xg[:, :, 3], OP.mult
                )
                nc.gpsimd.tensor_tensor(
                    q[:, :, 3, 0], xg[:, :, 0], xg[:, :, 1], OP.mult
                )
                # outer 3x3 block from the bf16 triples
                pab3 = (
                    q[:, :, 1:4, 0]
                    .unsqueeze(3)
                    .broadcast_to([TILE_B, CG, 3, 3])
                )
                pcd3 = (
                    q[:, :, 0, 1:4]
                    .unsqueeze(2)
                    .broadcast_to([TILE_B, CG, 3, 3])
                )
                nc.gpsimd.tensor_tensor(
                    q[:, :, 1:4, 1:4], pab3, pcd3, OP.mult
                )

                # ---- transpose: qt[m, c, b] = q[b, (c, m)] ----
                qt = tpool.tile([128, CHUNK, TILE_B], FP16, tag="qt")
                nc.sync.dma_start_transpose(
                    qt[:], q[:].rearrange("p cg i j -> p (cg i j)")
                )
                state[mi] = {"qt": qt, "s_t": s_t}

            def matmuls(mi):
                st = state[mi]
                qt = st["qt"]
                # two PSUM waves of 2 chunks each for finer bank recycling
                aw, bw = [], []
                for w in range(2):
                    a_t = pspool.tile(
                        [128, 2, 512], F32, tag=f"a{w}", name=f"a{w}"
                    )
                    b_t = pspool.tile(
                        [128, 2, 512], F32, tag=f"b{w}", name=f"b{w}"
                    )
                    for i in range(2):
                        c = 2 * w + i
                        nc.tensor.matmul(
                            b_t[:, i, 0:320],
                            qt[64:128, c, :],
                            c_sb[64:128, 320:640],
                            start=True,
                            stop=True,
                        )
                    for i in range(2):
                        c = 2 * w + i
                        nc.tensor.matmul(
                            a_t[:, i, 0:320],
                            qt[0:64, c, :],
                            c_sb[0:64, 0:320],
                            start=True,
                            stop=True,
                        )
                    aw.append(a_t)
                    bw.append(b_t)
                st["aw"], st["bw"] = aw, bw

            def back(mi):
                st = state.pop(mi)
                aw, bw, s_t = st["aw"], st["bw"], st["s_t"]
                gi, k = divmod(mi, GRP)

                # evacuate beta banks per wave with a wide Act copy (bf16)
                l1 = lpool.tile([TILE_B, CHUNK, 4, RU], BF16, tag="l1")
                for w in range(2):
                    bcp = bpool.tile(
                        [TILE_B, 2, 4, RU], F32, tag=f"bcp{w}", name="bcp"
                    )
                    nc.scalar.activation(
                        bcp[:],
                        bw[w][:, :, 0:320].rearrange(
                            "p c (g k) -> p c g k", g=4
                        ),
                        AF.Copy,
                    )
                    # level 1: alpha (PSUM, DVE-only reader) x bcp
                    nc.vector.tensor_tensor(
                        l1[:, 2 * w : 2 * w + 2],
                        aw[w][:, :, 0:320].rearrange(
                            "p c (g k) -> p c g k", g=4
                        ),
                        bcp[:],
                        OP.mult,
                    )
                l2 = lpool.tile([TILE_B, CHUNK, 2, RU], BF16, tag="l2")
                nc.vector.tensor_tensor(
                    l2[:], l1[:, :, 0:2], l1[:, :, 2:4], OP.mult
                )
                l3 = lpool.tile([TILE_B, CHUNK, RU], BF16, tag="l3")
                nc.vector.tensor_tensor(
                    l3[:], l2[:, :, 0], l2[:, :, 1], OP.mult
                )

                # ---- sum over rank (ru = u*10 + r) ----
                of = opool.tile([TILE_B, CHUNK, U], F32, tag="of")
                nc.vector.tensor_reduce(
                    of[:],
                    l3[:].rearrange("p c (u r) -> p c u r", r=R),
                    AX.X,
                    OP.add,
                )
                # ---- apply S, into the grouped store tile ----
                if k == 0:
                    state["ost"] = opool.tile(
                        [TILE_B, GRP, CHUNK, U], F32, tag="os", name="ost"
                    )
                os_ = state["ost"]
                nc.vector.tensor_tensor(
                    os_[:, k],
                    of[:],
                    s_t[:].unsqueeze(2).broadcast_to([TILE_B, CHUNK, U]),
                    OP.mult,
                )
                if k == GRP - 1:
                    nc.sync.dma_start(out=out[gi], in_=os_[:])

            # software-pipelined emission, fronts two macros ahead:
            #   front(m+1), back(m-1), matmuls(m)
            front(0)
            front(1)
            matmuls(0)
            for mi in range(1, N_MACRO):
                if mi + 1 < N_MACRO:
                    front(mi + 1)
                back(mi - 1)
                matmuls(mi)
            back(N_MACRO - 1)
    nc.finalize()
    return nc


def _pack_weights(kernel: np.ndarray):
    import ml_dtypes

    K = kernel.astype(np.float64)  # [2, R, F, U]
    C = np.zeros((128, 2 * 4 * RU), np.float64)
    bits = [(0, 0), (1, 0), (0, 1), (1, 1)]
    for g in range(NG):
        half = g // 4
        for i, (ba, bb) in enumerate(bits):
            for j, (bc, bd) in enumerate(bits):
                m = g * 16 + i * 4 + j
                coef = (
                    K[ba, :, 4 * g, :]
                    * K[bb, :, 4 * g + 1, :]
                    * K[bc, :, 4 * g + 2, :]
                    * K[bd, :, 4 * g + 3, :]
                )  # [R, U]
                col0 = half * 320 + (g % 4) * RU
                # ru = u*10 + r; 2^3 scale per group is unwound by the
                # 2^-48 inside the Sqrt scale (8 groups x 2^3 = 2^24, and
                # sqrt(2^-48) = 2^-24)
                C[m, col0 : col0 + RU] = coef.T.reshape(RU) * 8.0
    return C.astype(np.float16)


_NC_CACHE = {}


def kernel(X: np.ndarray, kernel: np.ndarray) -> np.ndarray:
    if "nc" not in _NC_CACHE:
        _NC_CACHE["nc"] = build_nc()
    nc = _NC_CACHE["nc"]
    C = _pack_weights(kernel)
    X = np.ascontiguousarray(X, dtype=np.float32)
    # row b of core = gi*2048 + k*512 + c*128 + p  ->  [gi, p, k, c, f]
    Xd = (
        X.reshape(N_CORES, N_GRP, GRP, CHUNK, TILE_B, F)
        .transpose(0, 1, 4, 2, 3, 5)
        .copy()
    )
    in_maps = []
    for c in range(N_CORES):
        in_maps.append({"X": Xd[c], "C": C})
    res = run_bass_kernel_spmd(nc, in_maps, core_ids=list(range(N_CORES)))
    outs = []
    for c in range(N_CORES):
        o = res.results[c]["out"]  # [N_GRP, TILE_B, GRP, CHUNK, U]
        outs.append(o.transpose(0, 2, 3, 1, 4).reshape(B_CORE, U))
    return np.concatenate(outs, axis=0).astype(np.float32)


if __name__ == "__main__":
    rng = np.random.default_rng(0)
    X = rng.standard_normal((B_FULL, F), dtype=np.float32)
    K = (rng.standard_normal((2, R, F, U)) * 0.24).astype(np.float32)
    y = kernel(X, K)
    print(y.shape, y.dtype, np.abs(y).max())
